# Initial kernel scaffold
#
"""Segment-mean pooling (AvgPoolingLayer / segment_reduce) on 8 Trainium2 cores.

Strategy
--------
segment_ids are sorted, so each segment occupies a contiguous row range.
Shard rows across 8 cores at segment boundaries (each segment lives on
exactly one core).  Per core, the segment-sum is computed as a chain of
one-hot matmuls on the PE:

    psum[block] += one_hot(ids_tile)^T @ feats_tile

where one_hot is built on the DVE from a precomputed "relative id" input
(id - block_base, or -1 for rows not in the block) compared against an
iota constant.  PSUM accumulates fp32 over a 128-segment block; the block
is then scaled by 1/count and DMA'd to the output slice.

Precision: feats are split on the host into hi/lo bf16 pairs
(x ~= hi + lo, residual ~2^-18 * |x|), interleaved as [N, 2, D].  Both
halves stream through the PE at bf16 rate (1 cycle/row vs 4 for fp32) in
a single N=512 matmul per tile and accumulate into one fp32 PSUM bank,
so total DMA bytes are unchanged (4 B/element) and PE time stays below
the HBM roofline.

DMA layout: rows are assigned to SBUF partitions chunk-wise
(partition p of a 2048-row chunk holds rows [16p, 16p+16)), which makes
every feats DMA a fully linear HBM read with 16 KiB contiguous packets
per partition — 1 KiB packets (row-interleaved layout) cap the 16 DMA
engines at ~290 GB/s, well under the ~358 GB/s HBM peak.  The row
permutation is absorbed into the precomputed rel inputs.

SPMD: one Bass program runs on all 8 cores; all per-core differences
(row windows, relative ids, inverse counts) are carried in the input
data, never in the instruction stream.
"""

import numpy as np
import ml_dtypes

from concourse import bass, mybir, tile
from concourse.bass_utils import run_bass_kernel_spmd

N = 1_000_000
D = 256
S = 10_000
NCORES = 8
P = 128           # rows per matmul tile == SBUF partitions
CHUNK = 16        # tiles per feats DMA == consecutive rows per partition
SPC = S // NCORES # segments owned per core
NBLK = (SPC + P - 1) // P  # 128-segment PSUM blocks per core

_f32 = mybir.dt.float32
_bf16 = mybir.dt.bfloat16


def _plan(ids, n_rows, n_cores, segs_per_core, nblk, chunk):
    """Host-side plan: per-core row windows + static (tile, block) issue list.

    Row order is partition-major within each P*chunk-row chunk: tile
    (c, n) covers rows {chunk_start + chunk*p + n : p in 0..P-1}.
    Returns (starts, R, issue, rel, first_slot, last_slot) where
    issue = [(t, b), ...] is the SPMD-static matmul schedule (union over
    cores of blocks touched by each tile) and rel is the per-core
    [P, n_slots] relative segment id array (-1 = no hit).
    """
    g = np.arange(n_cores + 1, dtype=np.int64) * segs_per_core
    b_rows = np.searchsorted(ids, g, side="left")
    spans = b_rows[1:] - b_rows[:-1]
    R = int(np.ceil(spans.max() / (P * chunk)) * (P * chunk))
    assert R <= n_rows and R >= spans.max()
    starts = np.minimum(b_rows[:-1], n_rows - R)
    T = R // P
    nchunk = T // chunk

    # per-core relative segment index of every row in its window,
    # reshaped to the partition-major tile order: [C, nchunk, P, chunk]
    vals = np.stack([ids[s:s + R] for s in starts]).astype(np.int64)
    vals -= g[:-1, None]
    vals_t = vals.reshape(n_cores, nchunk, P, chunk)
    owned = (vals_t >= 0) & (vals_t < segs_per_core)
    blk = np.where(owned, vals_t >> 7, -1)

    issue = []
    for c in range(nchunk):
        for n in range(chunk):
            bs = np.unique(blk[:, c, :, n])
            issue.extend((c * chunk + n, int(b)) for b in bs if b >= 0)

    n_slots = len(issue)
    rel = np.full((n_cores, P, n_slots), -1.0, dtype=np.float32)
    for i, (t, b) in enumerate(issue):
        v = vals_t[:, t // chunk, :, t % chunk] - b * P  # [C, P]
        hit = (v >= 0) & (v < P)
        rel[:, :, i] = np.where(hit, v, -1).astype(np.float32)

    first_slot, last_slot = {}, {}
    for i, (t, b) in enumerate(issue):
        first_slot.setdefault(b, i)
        last_slot[b] = i
    assert set(first_slot) == set(range(nblk)), (
        f"blocks missing from issue list: {sorted(set(range(nblk)) - set(first_slot))}"
    )
    return starts, R, issue, rel, first_slot, last_slot


def _build_program(R, d, nblk, issue, first_slot, last_slot, chunk):
    """Emit the SPMD Bass program (identical for all cores)."""
    T = R // P
    n_slots = len(issue)
    nc = bass.Bass()
    hilo_d = nc.dram_tensor("hilo", [R, 2, d], _bf16, kind="ExternalInput")
    # iota is bf16 (fast DVE input); rel must be f32 (tensor_scalar
    # scalar operand), packed with inv so one DMA covers both
    iota_d = nc.dram_tensor("iota", [P, P], _bf16, kind="ExternalInput")
    meta_d = nc.dram_tensor("meta", [P, n_slots + nblk], _f32,
                            kind="ExternalInput")
    out_d = nc.dram_tensor("out", [nblk * P, d], _f32, kind="ExternalOutput")

    with tile.TileContext(nc) as tc:
        with (
            tc.tile_pool(name="const", bufs=1) as cpool,
            tc.tile_pool(name="feats", bufs=5) as fpool,
            tc.tile_pool(name="oh", bufs=8) as ohpool,
            tc.tile_pool(name="acc", bufs=4, space=bass.MemorySpace.PSUM) as pspool,
            tc.tile_pool(name="res", bufs=nblk + 1) as rpool,
        ):
            iota_tile = cpool.tile([P, P], _bf16)
            nc.sync.dma_start(iota_tile[:], iota_d[:])
            meta_t = cpool.tile([P, n_slots + nblk], _f32)
            nc.sync.dma_start(meta_t[:], meta_d[:])
            iota_t = iota_tile[:]
            rel_t = meta_t[:, 0:n_slots]
            inv_t = meta_t[:, n_slots:]

            # PE warm-up: ~20 dummy matmuls while the first feats chunk is
            # in flight keep the HAM activity window busy so the PE clock
            # gate opens (1.2 -> 2.4 GHz) before real work arrives.
            warm = cpool.tile([P, P], _bf16, name="warm")
            nc.vector.memset(warm[:], 0.0)
            warm_rhs = cpool.tile([P, 2, d], _bf16, name="warm_rhs")
            nc.vector.memset(warm_rhs[:], 0.0)
            wacc = pspool.tile([P, 2, d], _f32, name="wacc", tag="acc")
            for _ in range(16):
                nc.tensor.matmul(wacc[:], warm[:], warm_rhs[:],
                                 start=True, stop=True)

            psum_tiles = {}
            pending = []  # (ready_slot, block, psum_tile)

            def emit_combine(b, pt):
                # combine hi+lo sums and scale by 1/count — all on DVE so
                # each op waits on at most one foreign semaphore (PE's
                # stop matmul).  The output DMA goes on the idle Scalar
                # engine's queue: on Sync it would head-of-line-block the
                # feats chunk loads behind the combine's completion.
                res = rpool.tile([P, d], _f32, name="res", tag="res")
                lo_sb = rpool.tile([P, d], _f32, name="lo_sb", tag="lo_sb")
                nc.vector.tensor_copy(lo_sb[:], pt[:, 1, :])
                nc.vector.tensor_tensor(
                    out=res[:], in0=pt[:, 0, :], in1=lo_sb[:],
                    op=mybir.AluOpType.add)
                nc.vector.tensor_scalar(
                    out=res[:], in0=res[:],
                    scalar1=inv_t[:, b:b + 1], scalar2=None,
                    op0=mybir.AluOpType.mult)
                nc.sync.dma_start(out_d[b * P:(b + 1) * P, :], res[:])

            COMBINE_DELAY = 0
            slot = 0
            for c in range(T // chunk):
                hl = fpool.tile([P, chunk, 2, d], _bf16)
                r0 = c * chunk * P
                src = hilo_d[r0:r0 + chunk * P].rearrange(
                    "(p n) two d -> p n two d", p=P)
                nc.sync.dma_start(hl[:], src)
                for j in range(chunk):
                    t = c * chunk + j
                    while slot < n_slots and issue[slot][0] == t:
                        b = issue[slot][1]
                        oh = ohpool.tile([P, P], _bf16)
                        nc.vector.tensor_scalar(
                            out=oh[:], in0=iota_t,
                            scalar1=rel_t[:, slot:slot + 1], scalar2=None,
                            op0=mybir.AluOpType.is_equal)
                        if b not in psum_tiles:
                            psum_tiles[b] = pspool.tile(
                                [P, 2, d], _f32, name="acc", tag="acc")
                        pt = psum_tiles[b]
                        nc.tensor.matmul(pt[:], oh[:], hl[:, j, :, :],
                                         start=(slot == first_slot[b]),
                                         stop=(slot == last_slot[b]))
                        if slot == last_slot[b]:
                            pending.append((slot + COMBINE_DELAY, b, pt))
                            del psum_tiles[b]
                        slot += 1
                        while pending and pending[0][0] <= slot:
                            _, pb, ppt = pending.pop(0)
                            emit_combine(pb, ppt)
            for _, pb, ppt in pending:
                emit_combine(pb, ppt)
    assert slot == n_slots
    _strip_self_waits(nc)
    _legalize_waits(nc)
    return nc


# Compute ops whose ISA structs carry a single sync-wait slot.  Tile's
# pool-slot release join sometimes adds a same-engine WAW/WAR wait on top
# of a cross-engine one; same-engine ordering is already guaranteed by
# in-order execution (Tile records same-engine deps as no-sync edges
# elsewhere), so the self-wait is redundant and safe to drop.
_COMPUTE_OPS = (
    mybir.InstTensorTensor, mybir.InstTensorScalarPtr,
    mybir.InstTensorCopy, mybir.InstActivation, mybir.InstMemset,
    mybir.InstMatmult, mybir.InstLdweights, mybir.InstTensorReduce,
)

_COMPUTE_SEMS = ("PE_", "DVE_", "Pool_", "Activation_", "SP_")


def _strip_self_waits(nc):
    for bb in nc.main_func.blocks:
        for ins in bb.instructions:
            si = ins.sync_info
            if si is None or not si.on_wait:
                continue
            if isinstance(ins, _COMPUTE_OPS):
                eng = str(ins.engine).split(".")[-1]
                kept = [w for w in si.on_wait
                        if not w.ant_name.startswith(eng + "_")]
                if len(kept) != len(si.on_wait):
                    si.on_wait = kept
            elif isinstance(ins, mybir.InstDMACopy) and len(si.on_wait) > 1:
                # A WAW wait on the old writer's DMA queue is implied by the
                # compute-engine wait that gates on the old tile's readers
                # (the readers FIFO-follow a wait on that very queue).
                has_compute = any(
                    w.ant_name.startswith(_COMPUTE_SEMS) for w in si.on_wait)
                if has_compute:
                    kept = [w for w in si.on_wait
                            if not w.ant_name.startswith("DMAHW")]
                    if kept and len(kept) != len(si.on_wait):
                        si.on_wait = kept


def _legalize_waits(nc, maxw=1):
    """The walrus codegen here supports very few sync-wait commands per
    instruction.  Hoist excess waits onto preceding same-engine NoOps —
    engine FIFO order makes this equivalent."""
    for bb in nc.main_func.blocks:
        idx = 0
        while idx < len(bb.instructions):
            ins = bb.instructions[idx]
            si = ins.sync_info
            if si is not None and si.on_wait and len(si.on_wait) > maxw:
                waits = list(si.on_wait)
                si.on_wait = waits[-maxw:]
                for w in waits[:-maxw]:
                    nop = mybir.InstNoOp(
                        name=nc.get_next_instruction_name(),
                        engine=ins.engine,
                        sync_info=mybir.SyncInfo(on_wait=[w], on_update=[]),
                        bass_nofuse=True,
                    )
                    bb.instructions.insert(idx, nop)
                    idx += 1
            idx += 1


def _prepare_inputs(feats, ids, n_cores, segs_per_core, nblk, starts, R, rel):
    """Per-core input maps: interleaved hi/lo bf16 feats + meta + inv."""
    n, d = feats.shape
    counts = np.bincount(ids, minlength=n_cores * segs_per_core).astype(np.float32)
    inv = (1.0 / np.maximum(counts, 1.0)).astype(np.float32)
    inv_pad = np.zeros(n_cores * segs_per_core + nblk * P, np.float32)
    inv_pad[:inv.shape[0]] = inv

    hi = feats.astype(ml_dtypes.bfloat16)
    lo = (feats - hi.astype(np.float32)).astype(ml_dtypes.bfloat16)
    hilo = np.empty((n, 2, d), dtype=ml_dtypes.bfloat16)
    hilo[:, 0, :] = hi
    hilo[:, 1, :] = lo

    n_slots = rel.shape[2]
    # iota[p, j] = j — compared against rel[p] to build the one-hot
    iota = np.broadcast_to(np.arange(P, dtype=np.float32), (P, P))
    in_maps = []
    for c in range(n_cores):
        g0 = c * segs_per_core
        inv_c = inv_pad[g0:g0 + nblk * P].copy()
        inv_c[segs_per_core:] = 0.0
        meta = np.empty((P, n_slots + nblk), np.float32)
        meta[:, 0:n_slots] = rel[c]
        meta[:, n_slots:] = inv_c.reshape(nblk, P).T
        in_maps.append({
            "hilo": hilo[starts[c]:starts[c] + R],
            "iota": iota.astype(ml_dtypes.bfloat16),
            "meta": meta,
        })
    return in_maps


def _run(feats, ids, n_cores, segs_per_core, nblk, chunk, trace=False,
         trace_cores=None):
    n, d = feats.shape
    starts, R, issue, rel, first_slot, last_slot = _plan(
        ids, n, n_cores, segs_per_core, nblk, chunk)
    nc = _build_program(R, d, nblk, issue, first_slot, last_slot, chunk)
    in_maps = _prepare_inputs(feats, ids, n_cores, segs_per_core, nblk,
                              starts, R, rel)
    res = run_bass_kernel_spmd(nc, in_maps, list(range(n_cores)),
                               trace=trace, trace_cores=trace_cores)
    out = np.concatenate(
        [res.results[c]["out"][:segs_per_core] for c in range(n_cores)], axis=0)
    return out, res


def kernel(feats, segment_ids, num_segments):
    feats = np.ascontiguousarray(np.asarray(feats), dtype=np.float32)
    ids = np.asarray(segment_ids).astype(np.int64)
    s = int(num_segments)
    assert feats.shape == (N, D) and ids.shape == (N,) and s == S, (
        "kernel is specialized for feats [1e6, 256], 1e4 segments")
    out, _ = _run(feats, ids, NCORES, SPC, NBLK, CHUNK)
    return out



# revision 1
# speedup vs baseline: 2.6179x; 2.6179x over previous
"""Segment-mean pooling (AvgPoolingLayer / segment_reduce) on 8 Trainium2 cores.

Strategy
--------
segment_ids are sorted, so each segment occupies a contiguous row range.
Shard rows across 8 cores at segment boundaries (each segment lives on
exactly one core).  Per core, the segment-sum is computed as a chain of
one-hot matmuls on the PE:

    psum[block] += one_hot(ids_tile)^T @ feats_tile

where one_hot is built on the DVE from a precomputed "relative id" input
(id - block_base, or -1 for rows not in the block) compared against an
iota constant.  PSUM accumulates fp32 over a 128-segment block; the block
is then scaled by 1/count and DMA'd to the output slice.

Precision: feats are split on the host into hi/lo bf16 pairs
(x ~= hi + lo, residual ~2^-18 * |x|), interleaved as [N, 2, D].  Both
halves stream through the PE at bf16 rate (1 cycle/row vs 4 for fp32) in
a single N=512 matmul per tile and accumulate into one fp32 PSUM bank,
so total DMA bytes are unchanged (4 B/element) and PE time stays below
the HBM roofline.

DMA layout: rows are assigned to SBUF partitions chunk-wise
(partition p of a 2048-row chunk holds rows [16p, 16p+16)), which makes
every feats DMA a fully linear HBM read with 16 KiB contiguous packets
per partition — 1 KiB packets (row-interleaved layout) cap the 16 DMA
engines at ~290 GB/s, well under the ~358 GB/s HBM peak.  The row
permutation is absorbed into the precomputed rel inputs.

SPMD: one Bass program runs on all 8 cores; all per-core differences
(row windows, relative ids, inverse counts) are carried in the input
data, never in the instruction stream.
"""

import numpy as np
import ml_dtypes

from concourse import bass, mybir, tile
from concourse.bass_utils import run_bass_kernel_spmd

N = 1_000_000
D = 256
S = 10_000
NCORES = 8
P = 128           # rows per matmul tile == SBUF partitions
CHUNK = 16        # tiles per feats DMA == consecutive rows per partition
SPC = S // NCORES # segments owned per core
NBLK = (SPC + P - 1) // P  # 128-segment PSUM blocks per core

_f32 = mybir.dt.float32
_bf16 = mybir.dt.bfloat16


def _plan(ids, n_rows, n_cores, segs_per_core, nblk, chunk):
    """Host-side plan: per-core row windows + static (tile, block) issue list.

    Row order is partition-major within each P*chunk-row chunk: tile
    (c, n) covers rows {chunk_start + chunk*p + n : p in 0..P-1}.
    Returns (starts, R, issue, rel, first_slot, last_slot) where
    issue = [(t, b), ...] is the SPMD-static matmul schedule (union over
    cores of blocks touched by each tile) and rel is the per-core
    [P, n_slots] relative segment id array (-1 = no hit).
    """
    g = np.arange(n_cores + 1, dtype=np.int64) * segs_per_core
    b_rows = np.searchsorted(ids, g, side="left")
    spans = b_rows[1:] - b_rows[:-1]
    R = int(np.ceil(spans.max() / (P * chunk)) * (P * chunk))
    assert R <= n_rows and R >= spans.max()
    starts = np.minimum(b_rows[:-1], n_rows - R)
    T = R // P
    nchunk = T // chunk

    # per-core relative segment index of every row in its window,
    # reshaped to the partition-major tile order: [C, nchunk, P, chunk]
    vals = np.stack([ids[s:s + R] for s in starts]).astype(np.int64)
    vals -= g[:-1, None]
    vals_t = vals.reshape(n_cores, nchunk, P, chunk)
    owned = (vals_t >= 0) & (vals_t < segs_per_core)
    blk = np.where(owned, vals_t >> 7, -1)

    issue = []
    for c in range(nchunk):
        for n in range(chunk):
            bs = np.unique(blk[:, c, :, n])
            issue.extend((c * chunk + n, int(b)) for b in bs if b >= 0)

    n_slots = len(issue)
    rel = np.full((n_cores, P, n_slots), -1.0, dtype=np.float32)
    for i, (t, b) in enumerate(issue):
        v = vals_t[:, t // chunk, :, t % chunk] - b * P  # [C, P]
        hit = (v >= 0) & (v < P)
        rel[:, :, i] = np.where(hit, v, -1).astype(np.float32)

    first_slot, last_slot = {}, {}
    for i, (t, b) in enumerate(issue):
        first_slot.setdefault(b, i)
        last_slot[b] = i
    assert set(first_slot) == set(range(nblk)), (
        f"blocks missing from issue list: {sorted(set(range(nblk)) - set(first_slot))}"
    )
    return starts, R, issue, rel, first_slot, last_slot


def _build_program(R, d, nblk, issue, first_slot, last_slot, chunk):
    """Emit the SPMD Bass program (identical for all cores)."""
    T = R // P
    n_slots = len(issue)
    nc = bass.Bass()
    hilo_d = nc.dram_tensor("hilo", [R, 2, d], _bf16, kind="ExternalInput")
    # iota is bf16 (fast DVE input); rel must be f32 (tensor_scalar
    # scalar operand), packed with inv so one DMA covers both
    iota_d = nc.dram_tensor("iota", [P, P], _bf16, kind="ExternalInput")
    meta_d = nc.dram_tensor("meta", [P, n_slots + nblk], _f32,
                            kind="ExternalInput")
    out_d = nc.dram_tensor("out", [nblk * P, d], _f32, kind="ExternalOutput")

    with tile.TileContext(nc) as tc:
        with (
            tc.tile_pool(name="const", bufs=1) as cpool,
            tc.tile_pool(name="feats", bufs=5) as fpool,
            tc.tile_pool(name="oh", bufs=8) as ohpool,
            tc.tile_pool(name="acc", bufs=4, space=bass.MemorySpace.PSUM) as pspool,
            tc.tile_pool(name="res", bufs=nblk + 1) as rpool,
        ):
            iota_tile = cpool.tile([P, P], _bf16)
            nc.sync.dma_start(iota_tile[:], iota_d[:])
            meta_t = cpool.tile([P, n_slots + nblk], _f32)
            nc.sync.dma_start(meta_t[:], meta_d[:])
            iota_t = iota_tile[:]
            rel_t = meta_t[:, 0:n_slots]
            inv_t = meta_t[:, n_slots:]

            # PE warm-up: ~20 dummy matmuls while the first feats chunk is
            # in flight keep the HAM activity window busy so the PE clock
            # gate opens (1.2 -> 2.4 GHz) before real work arrives.
            warm = cpool.tile([P, P], _bf16, name="warm")
            nc.vector.memset(warm[:], 0.0)
            warm_rhs = cpool.tile([P, 2, d], _bf16, name="warm_rhs")
            nc.vector.memset(warm_rhs[:], 0.0)
            wacc = pspool.tile([P, 2, d], _f32, name="wacc", tag="acc")
            for _ in range(16):
                nc.tensor.matmul(wacc[:], warm[:], warm_rhs[:],
                                 start=True, stop=True)

            psum_tiles = {}
            pending = []  # (ready_slot, block, psum_tile)

            def emit_combine(b, pt):
                # combine hi+lo sums and scale by 1/count — all on DVE so
                # each op waits on at most one foreign semaphore (PE's
                # stop matmul).  The output DMA goes on the idle Scalar
                # engine's queue: on Sync it would head-of-line-block the
                # feats chunk loads behind the combine's completion.
                res = rpool.tile([P, d], _f32, name="res", tag="res")
                lo_sb = rpool.tile([P, d], _f32, name="lo_sb", tag="lo_sb")
                nc.vector.tensor_copy(lo_sb[:], pt[:, 1, :])
                nc.vector.tensor_tensor(
                    out=res[:], in0=pt[:, 0, :], in1=lo_sb[:],
                    op=mybir.AluOpType.add)
                nc.vector.tensor_scalar(
                    out=res[:], in0=res[:],
                    scalar1=inv_t[:, b:b + 1], scalar2=None,
                    op0=mybir.AluOpType.mult)
                nc.sync.dma_start(out_d[b * P:(b + 1) * P, :], res[:])

            COMBINE_DELAY = 0
            slot = 0
            for c in range(T // chunk):
                hl = fpool.tile([P, chunk, 2, d], _bf16)
                r0 = c * chunk * P
                src = hilo_d[r0:r0 + chunk * P].rearrange(
                    "(p n) two d -> p n two d", p=P)
                nc.sync.dma_start(hl[:], src)
                for j in range(chunk):
                    t = c * chunk + j
                    while slot < n_slots and issue[slot][0] == t:
                        b = issue[slot][1]
                        oh = ohpool.tile([P, P], _bf16)
                        nc.vector.tensor_scalar(
                            out=oh[:], in0=iota_t,
                            scalar1=rel_t[:, slot:slot + 1], scalar2=None,
                            op0=mybir.AluOpType.is_equal)
                        if b not in psum_tiles:
                            psum_tiles[b] = pspool.tile(
                                [P, 2, d], _f32, name="acc", tag="acc")
                        pt = psum_tiles[b]
                        nc.tensor.matmul(pt[:], oh[:], hl[:, j, :, :],
                                         start=(slot == first_slot[b]),
                                         stop=(slot == last_slot[b]))
                        if slot == last_slot[b]:
                            pending.append((slot + COMBINE_DELAY, b, pt))
                            del psum_tiles[b]
                        slot += 1
                        while pending and pending[0][0] <= slot:
                            _, pb, ppt = pending.pop(0)
                            emit_combine(pb, ppt)
            for _, pb, ppt in pending:
                emit_combine(pb, ppt)
    assert slot == n_slots
    _strip_self_waits(nc)
    _legalize_waits(nc)
    return nc


# Compute ops whose ISA structs carry a single sync-wait slot.  Tile's
# pool-slot release join sometimes adds a same-engine WAW/WAR wait on top
# of a cross-engine one; same-engine ordering is already guaranteed by
# in-order execution (Tile records same-engine deps as no-sync edges
# elsewhere), so the self-wait is redundant and safe to drop.
_COMPUTE_OPS = (
    mybir.InstTensorTensor, mybir.InstTensorScalarPtr,
    mybir.InstTensorCopy, mybir.InstActivation, mybir.InstMemset,
    mybir.InstMatmult, mybir.InstLdweights, mybir.InstTensorReduce,
)

_COMPUTE_SEMS = ("PE_", "DVE_", "Pool_", "Activation_", "SP_")


def _strip_self_waits(nc):
    for bb in nc.main_func.blocks:
        for ins in bb.instructions:
            si = ins.sync_info
            if si is None or not si.on_wait:
                continue
            if isinstance(ins, _COMPUTE_OPS):
                eng = str(ins.engine).split(".")[-1]
                kept = [w for w in si.on_wait
                        if not w.ant_name.startswith(eng + "_")]
                if len(kept) != len(si.on_wait):
                    si.on_wait = kept
            elif isinstance(ins, mybir.InstDMACopy) and len(si.on_wait) > 1:
                # A WAW wait on the old writer's DMA queue is implied by the
                # compute-engine wait that gates on the old tile's readers
                # (the readers FIFO-follow a wait on that very queue).
                has_compute = any(
                    w.ant_name.startswith(_COMPUTE_SEMS) for w in si.on_wait)
                if has_compute:
                    kept = [w for w in si.on_wait
                            if not w.ant_name.startswith("DMAHW")]
                    if kept and len(kept) != len(si.on_wait):
                        si.on_wait = kept


def _legalize_waits(nc, maxw=1):
    """The walrus codegen here supports very few sync-wait commands per
    instruction.  Hoist excess waits onto preceding same-engine NoOps —
    engine FIFO order makes this equivalent."""
    for bb in nc.main_func.blocks:
        idx = 0
        while idx < len(bb.instructions):
            ins = bb.instructions[idx]
            si = ins.sync_info
            if si is not None and si.on_wait and len(si.on_wait) > maxw:
                waits = list(si.on_wait)
                si.on_wait = waits[-maxw:]
                for w in waits[:-maxw]:
                    nop = mybir.InstNoOp(
                        name=nc.get_next_instruction_name(),
                        engine=ins.engine,
                        sync_info=mybir.SyncInfo(on_wait=[w], on_update=[]),
                        bass_nofuse=True,
                    )
                    bb.instructions.insert(idx, nop)
                    idx += 1
            idx += 1


def _prepare_inputs(feats, ids, n_cores, segs_per_core, nblk, starts, R, rel):
    """Per-core input maps: interleaved hi/lo bf16 feats + meta + inv."""
    n, d = feats.shape
    counts = np.bincount(ids, minlength=n_cores * segs_per_core).astype(np.float32)
    inv = (1.0 / np.maximum(counts, 1.0)).astype(np.float32)
    inv_pad = np.zeros(n_cores * segs_per_core + nblk * P, np.float32)
    inv_pad[:inv.shape[0]] = inv

    hi = feats.astype(ml_dtypes.bfloat16)
    lo = (feats - hi.astype(np.float32)).astype(ml_dtypes.bfloat16)
    hilo = np.empty((n, 2, d), dtype=ml_dtypes.bfloat16)
    hilo[:, 0, :] = hi
    hilo[:, 1, :] = lo

    n_slots = rel.shape[2]
    # iota[p, j] = j — compared against rel[p] to build the one-hot
    iota = np.broadcast_to(np.arange(P, dtype=np.float32), (P, P))
    in_maps = []
    for c in range(n_cores):
        g0 = c * segs_per_core
        inv_c = inv_pad[g0:g0 + nblk * P].copy()
        inv_c[segs_per_core:] = 0.0
        meta = np.empty((P, n_slots + nblk), np.float32)
        meta[:, 0:n_slots] = rel[c]
        meta[:, n_slots:] = inv_c.reshape(nblk, P).T
        in_maps.append({
            "hilo": hilo[starts[c]:starts[c] + R],
            "iota": iota.astype(ml_dtypes.bfloat16),
            "meta": meta,
        })
    return in_maps


def _run(feats, ids, n_cores, segs_per_core, nblk, chunk, trace=False,
         trace_cores=None):
    n, d = feats.shape
    starts, R, issue, rel, first_slot, last_slot = _plan(
        ids, n, n_cores, segs_per_core, nblk, chunk)
    nc = _build_program(R, d, nblk, issue, first_slot, last_slot, chunk)
    in_maps = _prepare_inputs(feats, ids, n_cores, segs_per_core, nblk,
                              starts, R, rel)
    res = run_bass_kernel_spmd(nc, in_maps, list(range(n_cores)),
                               trace=trace, trace_cores=trace_cores)
    out = np.concatenate(
        [res.results[c]["out"][:segs_per_core] for c in range(n_cores)], axis=0)
    return out, res


def kernel(feats, segment_ids, num_segments):
    feats = np.ascontiguousarray(np.asarray(feats), dtype=np.float32)
    ids = np.asarray(segment_ids).astype(np.int64)
    s = int(num_segments)
    assert feats.shape == (N, D) and ids.shape == (N,) and s == S, (
        "kernel is specialized for feats [1e6, 256], 1e4 segments")
    out, _ = _run(feats, ids, NCORES, SPC, NBLK, CHUNK)
    return out

